# revision 22
# baseline (speedup 1.0000x reference)
"""Trainium2 Bass kernel for BackboneMultiviewGmCoPo.

Math: the 4D bilinear upsample of the coarse cost volume is linear in the
features, so  c = 0.5*(up4d(corr_c) + corr_f) = a * G0^T G1  over stacked
256-channel features G = [resize(F_c); F_f], a = 0.5/sqrt(128).
Sharding: 8 cores = (batch b in {0,1}) x (quarter kq in {0..3}) of the 4096
rows.  Per core, task A computes M[p_slice, :] (c rows + row-softmax ->
flow), task B computes M^T[q_slice, :] (row-softmax -> flow_flip).
c_flip is a host-side transpose view of c; trans_features is a host restack
of the inputs; trans_features_coarse is the (host) bilinear resize that also
feeds the matmul.
"""
import numpy as np
import ml_dtypes

import concourse.bass as bass
import concourse.mybir as mybir
import concourse.tile as tile
from concourse.bass_utils import run_bass_kernel_spmd
from concourse.vector_clock import ScopedClock

# ---------------------------------------------------------------------------
# Walrus-compat shims (this container's walrus accepts at most ONE sem wait
# per instruction): split the Tile tail-drain's waits across single-wait
# Drains, and hoist excess waits of any instruction onto NoOp carriers.
_MAXW = 1
_wsn = [0]


def _patched_drain_and_barrier(self, tick_clock, wait_clock):
    drain_inst = self.nc.sync.drain()
    wait_clock.add_sem_waits(drain_inst.ins, ScopedClock({None: tick_clock.global_clock}))
    si = drain_inst.ins.sync_info
    waits = list(si.on_wait) if si is not None else []
    if len(waits) > _MAXW:
        drain_inst.ins.sync_info = mybir.SyncInfo(on_wait=waits[:_MAXW], on_update=[])
        for i in range(_MAXW, len(waits), _MAXW):
            extra = self.nc.sync.drain()
            extra.ins.sync_info = mybir.SyncInfo(on_wait=waits[i : i + _MAXW], on_update=[])
    self.nc.all_engine_barrier()
    assert self.sems is not None
    popped = self.nc._tile_sem_poison_stack.pop()
    assert popped is self._sem_poison
    self.nc.clear_and_free_semaphores(list(self.sems.allocated().values()))
    self.nc.all_engine_barrier()


tile.TileContext._drain_and_barrier = _patched_drain_and_barrier


def _split_multi_waits(nc):
    NoOp = getattr(mybir, "InstNoOp", None)
    if NoOp is None:
        import bass_rust
        NoOp = bass_rust.InstNoOp
    for fn in nc.m.functions:
        for bb in fn.blocks:
            if not any(
                i.sync_info is not None and len(i.sync_info.on_wait) > _MAXW
                for i in bb.instructions
            ):
                continue
            new_insts = []
            for i in bb.instructions:
                si = i.sync_info
                if si is not None and len(si.on_wait) > _MAXW:
                    waits = list(si.on_wait)
                    for w in waits[:-_MAXW]:
                        _wsn[0] += 1
                        new_insts.append(NoOp(
                            name=f"WSPLIT-{_wsn[0]}", engine=i.engine, ins=[], outs=[],
                            sync_info=mybir.SyncInfo(on_wait=[w], on_update=[]),
                        ))
                    i.sync_info = mybir.SyncInfo(
                        on_wait=waits[-_MAXW:], on_update=list(si.on_update)
                    )
                new_insts.append(i)
            bb.instructions = new_insts
# ---------------------------------------------------------------------------

F32 = mybir.dt.float32
F32R = mybir.dt.float32r
BF16 = mybir.dt.bfloat16
AOP = mybir.AluOpType
AFT = mybir.ActivationFunctionType

B, C, S = 2, 128, 64
NCORES = 8
NQ = S * S            # 4096
NCHUNK = 8            # 128-row chunks per task
HALF = 2048           # free-dim half-chunk (4 PSUM banks)
BETA = 0.02
INV_BETA = 1.0 / BETA  # 50
MARGIN = 0.0          # exact per-half row max → no margin needed
SCALE = float(np.sqrt(0.5 / np.sqrt(128.0)))

_cache = {}


def _resize_ac64(x):
    """Bilinear align_corners resize of last two dims 32x32 -> 64x64 (numpy)."""
    H = x.shape[-1]
    ys = np.linspace(0.0, H - 1.0, 64).astype(np.float32)
    y0 = np.floor(ys).astype(np.int64)
    y1 = np.minimum(y0 + 1, H - 1)
    wy = (ys - y0).astype(np.float32)
    rows = x[..., y0, :] * (1.0 - wy)[:, None] + x[..., y1, :] * wy[:, None]
    return rows[..., y0] * (1.0 - wy) + rows[..., y1] * wy


def _build_nc():
    import os
    mode = os.environ.get("BMGC_MODE", "full")  # full | nodma | dmaonly
    nc = bass.Bass(trn_type="TRN2")
    d = {}
    d["lhsA"] = nc.dram_tensor("lhsA", [2, 128, 1024], F32R, kind="ExternalInput")
    d["rhsA"] = nc.dram_tensor("rhsA", [2, 128, NQ], F32R, kind="ExternalInput")
    d["lhsB"] = nc.dram_tensor("lhsB", [2, 128, 1024], F32R, kind="ExternalInput")
    d["rhsB"] = nc.dram_tensor("rhsB", [2, 128, NQ], F32R, kind="ExternalInput")
    d["wx"] = nc.dram_tensor("wx", [128, NQ], BF16, kind="ExternalInput")
    d["wy"] = nc.dram_tensor("wy", [128, NQ], BF16, kind="ExternalInput")
    d["bx8"] = nc.dram_tensor("bx8", [128, NCHUNK], F32, kind="ExternalInput")
    d["by8"] = nc.dram_tensor("by8", [128, NCHUNK], F32, kind="ExternalInput")
    # host-computed per-row softmax offsets, pre-scaled: -50 * mhat  [task,128,chunk]
    d["negm"] = nc.dram_tensor("negm", [2, 128, NCHUNK], F32, kind="ExternalInput")
    d["c_out"] = nc.dram_tensor("c_out", [NCHUNK, 128, NQ], F32, kind="ExternalOutput")
    d["fxy"] = nc.dram_tensor("fxy", [2, 2, 128, NCHUNK], F32, kind="ExternalOutput")

    with tile.TileContext(nc) as tc:
        with (
            tc.tile_pool(name="const", bufs=1) as constp,
            tc.tile_pool(name="mats", bufs=1) as matp,
            tc.tile_pool(name="msb", bufs=3) as msbp,
            tc.tile_pool(name="ep", bufs=5) as ep,
            tc.tile_pool(name="scr", bufs=2) as scrp,
            tc.tile_pool(name="stat", bufs=2) as statp,
            tc.tile_pool(name="ps", bufs=2, space="PSUM") as psp,
        ):
            wx_t = constp.tile([128, NQ], BF16)
            wy_t = constp.tile([128, NQ], BF16)
            bx_t = constp.tile([128, NCHUNK], F32)
            by_t = constp.tile([128, NCHUNK], F32)
            negm_t = constp.tile([128, 2, NCHUNK], F32)
            nc.sync.dma_start(wx_t[:], d["wx"][:])
            nc.sync.dma_start(wy_t[:], d["wy"][:])
            nc.sync.dma_start(bx_t[:], d["bx8"][:])
            nc.sync.dma_start(by_t[:], d["by8"][:])
            nc.sync.dma_start(negm_t[:], d["negm"].rearrange("t p c -> p t c"))

            lhs_t = {}
            rhs_t = {}
            for T, lname, rname in (("A", "lhsA", "rhsA"), ("B", "lhsB", "rhsB")):
                lt = matp.tile([128, 2, 1024], F32R, tag=f"lhs{T}")
                rt = matp.tile([128, 2, NQ], F32R, tag=f"rhs{T}")
                for k in range(2):
                    nc.sync.dma_start(lt[:, k], d[lname][k])
                    nc.sync.dma_start(rt[:, k], d[rname][k])
                lhs_t[T], rhs_t[T] = lt, rt

            for ti, T in enumerate(("A", "B")):
                m_a = statp.tile([128, NCHUNK], F32, tag=f"ma{T}")
                m_b = statp.tile([128, NCHUNK], F32, tag=f"mb{T}")
                mc = statp.tile([128, NCHUNK], F32, tag=f"mc{T}")
                negm = statp.tile([128, NCHUNK], F32, tag=f"ng{T}")
                D_h = statp.tile([128, 16], F32, tag=f"D{T}")
                Nx_h = statp.tile([128, 16], F32, tag=f"Nx{T}")
                Ny_h = statp.tile([128, 16], F32, tag=f"Ny{T}")
                def emit_marginals(h, e_t):
                    half = h % 2
                    scx = scrp.tile([128, HALF], BF16, tag="scx")
                    nc.vector.scalar_tensor_tensor(
                        out=scx[:], in0=e_t[:], scalar=1.0,
                        in1=wx_t[:, half * HALF : (half + 1) * HALF],
                        op0=AOP.mult, op1=AOP.mult,
                        accum_out=Nx_h[:, h : h + 1],
                    )
                    scy = scrp.tile([128, HALF], BF16, tag="scy")
                    nc.vector.scalar_tensor_tensor(
                        out=scy[:], in0=e_t[:], scalar=1.0,
                        in1=wy_t[:, half * HALF : (half + 1) * HALF],
                        op0=AOP.mult, op1=AOP.mult,
                        accum_out=Ny_h[:, h : h + 1],
                    )

                pend = []  # deferred (h, e_t) marginals (one chunk of delay)
                for chunk in range(NCHUNK):
                    phs = []
                    esrcs = []
                    for half in range(2):
                        ph = psp.tile([128, HALF], F32, tag="ph")
                        lhs_s = lhs_t[T][:, :, chunk * 128 : (chunk + 1) * 128]
                        for k in range(2):
                            for bank in range(4):
                                q0 = half * HALF + bank * 512
                                nc.tensor.matmul(
                                    ph[:, bank * 512 : (bank + 1) * 512],
                                    lhs_s[:, k],
                                    rhs_t[T][:, k, q0 : q0 + 512],
                                    start=(k == 0),
                                    stop=(k == 1),
                                )
                        phs.append(ph)
                        # per-half exact row max from PSUM
                        mdst = (m_a if half == 0 else m_b)[:, chunk : chunk + 1]
                        nc.vector.tensor_reduce(
                            out=mdst, in_=ph[:], axis=mybir.AxisListType.X, op=AOP.max,
                        )
                        msb = msbp.tile([128, HALF], F32, tag="msb")
                        nc.scalar.activation(
                            msb[:], ph[:], AFT.Copy, bias=0.0, scale=1.0
                        )
                        if T == "A":
                            nc.sync.dma_start(
                                d["c_out"][chunk, :, half * HALF : (half + 1) * HALF],
                                msb[:],
                            )
                        esrcs.append(msb)
                    # one offset per chunk = max of the two halves, pre-scaled
                    nc.vector.tensor_tensor(
                        out=mc[:, chunk : chunk + 1],
                        in0=m_a[:, chunk : chunk + 1], in1=m_b[:, chunk : chunk + 1],
                        op=AOP.max,
                    )
                    nc.scalar.activation(
                        negm[:, chunk : chunk + 1], mc[:, chunk : chunk + 1],
                        AFT.Copy, bias=0.0, scale=-INV_BETA,
                    )
                    for half in range(2):
                        h = chunk * 2 + half
                        e_t = ep.tile([128, HALF], BF16, tag="e")
                        nc.scalar.activation(
                            e_t[:], esrcs[half][:], AFT.Exp,
                            bias=negm[:, chunk : chunk + 1], scale=INV_BETA,
                            accum_out=D_h[:, h : h + 1],
                        )
                        pend.append((h, e_t))
                    while len(pend) > 2:
                        emit_marginals(*pend.pop(0))
                for item in pend:
                    emit_marginals(*item)

                # ---- per-task epilogue: combine halves (same offset per chunk),
                # softargmax -> flow
                ev = lambda t: t[:].rearrange("p (c h) -> p c h", h=2)[:, :, 0]
                od = lambda t: t[:].rearrange("p (c h) -> p c h", h=2)[:, :, 1]
                Dc = statp.tile([128, NCHUNK], F32, tag=f"dc{T}")
                Nxc = statp.tile([128, NCHUNK], F32, tag=f"nxc{T}")
                Nyc = statp.tile([128, NCHUNK], F32, tag=f"nyc{T}")
                nc.vector.tensor_tensor(out=Dc[:], in0=ev(D_h), in1=od(D_h), op=AOP.add)
                nc.vector.tensor_tensor(out=Nxc[:], in0=ev(Nx_h), in1=od(Nx_h), op=AOP.add)
                nc.vector.tensor_tensor(out=Nyc[:], in0=ev(Ny_h), in1=od(Ny_h), op=AOP.add)
                rD = statp.tile([128, NCHUNK], F32, tag=f"rd{T}")
                nc.vector.reciprocal(rD[:], Dc[:])
                fx = statp.tile([128, NCHUNK], F32, tag=f"fx{T}")
                fy = statp.tile([128, NCHUNK], F32, tag=f"fy{T}")
                tx = statp.tile([128, NCHUNK], F32, tag=f"tx{T}")
                ty = statp.tile([128, NCHUNK], F32, tag=f"ty{T}")
                nc.vector.scalar_tensor_tensor(
                    out=tx[:], in0=Nxc[:], scalar=31.5, in1=rD[:],
                    op0=AOP.mult, op1=AOP.mult)
                nc.vector.scalar_tensor_tensor(
                    out=ty[:], in0=Nyc[:], scalar=31.5, in1=rD[:],
                    op0=AOP.mult, op1=AOP.mult)
                nc.vector.tensor_tensor(out=fx[:], in0=tx[:], in1=bx_t[:], op=AOP.add)
                nc.vector.tensor_tensor(out=fy[:], in0=ty[:], in1=by_t[:], op=AOP.add)
                nc.sync.dma_start(d["fxy"][ti, 0], fx[:])
                nc.sync.dma_start(d["fxy"][ti, 1], fy[:])

    _split_multi_waits(nc)
    return nc


def _get_nc():
    if "nc" not in _cache:
        _cache["nc"] = _build_nc()
    return _cache["nc"]


def _host_consts():
    if "consts" in _cache:
        return _cache["consts"]
    xn = np.linspace(-1.0, 1.0, S).astype(np.float32)
    q = np.arange(NQ)
    wxq = (xn[q % S] + 1.0).astype(np.float32)          # shift +1
    wyq = (xn[q // S] + 2.0).astype(np.float32)         # shift +2
    wx = np.broadcast_to(wxq, (128, NQ)).astype(ml_dtypes.bfloat16)
    wy = np.broadcast_to(wyq, (128, NQ)).astype(ml_dtypes.bfloat16)
    p = np.arange(128)
    bx8 = np.broadcast_to((-(p % 64)).astype(np.float32)[:, None], (128, NCHUNK)).copy()
    _cache["consts"] = (wx, wy, bx8)
    return _cache["consts"]


def _row_offsets(ga, gb):
    """Softmax offset estimate per row of M = ga^T gb (ga, gb: [256, 4096]).
    Any value within (truemax - 1.7, truemax + 1.7) is numerically safe; build
    it from the correlated diagonal band + a stride-64 sample + a statistical
    row bound, all input-adaptive."""
    ga64 = ga.astype(np.float64)
    gb64 = gb.astype(np.float64)
    n = ga.shape[1]
    band = np.full(n, -np.inf)
    for dd in (0, 1, -1, 63, 64, 65, -63, -64, -65):
        if dd >= 0:
            v = (ga64[:, : n - dd] * gb64[:, dd:]).sum(0)
            band[: n - dd] = np.maximum(band[: n - dd], v)
        else:
            v = (ga64[:, -dd:] * gb64[:, : n + dd]).sum(0)
            band[-dd:] = np.maximum(band[-dd:], v)
    subm = (ga64.T @ gb64[:, ::64]).max(axis=1)
    sigma = np.sqrt((ga64 * ga64).sum(0)) * np.sqrt((gb64 * gb64).sum(0).mean() / ga.shape[0])
    est = 3.5 * sigma
    return np.maximum(np.maximum(band, subm), est).astype(np.float32)


def kernel(feat0_c, feat1_c, feat0_f, feat1_f):
    feat0_c = np.asarray(feat0_c, dtype=np.float32)
    feat1_c = np.asarray(feat1_c, dtype=np.float32)
    feat0_f = np.asarray(feat0_f, dtype=np.float32)
    feat1_f = np.asarray(feat1_f, dtype=np.float32)

    f0c_up = _resize_ac64(feat0_c)                      # [B,C,64,64]
    f1c_up = _resize_ac64(feat1_c)
    g0 = np.concatenate([f0c_up, feat0_f], axis=1) * SCALE   # [B,256,64,64]
    g1 = np.concatenate([f1c_up, feat1_f], axis=1) * SCALE
    g0 = np.ascontiguousarray(g0.reshape(B, 2, 128, NQ), dtype=np.float32)
    g1 = np.ascontiguousarray(g1.reshape(B, 2, 128, NQ), dtype=np.float32)

    wx, wy, bx8 = _host_consts()
    mhat = np.empty((B, 2, 4096), dtype=np.float32)
    for b in range(B):
        g0f = g0[b].reshape(256, NQ)
        g1f = g1[b].reshape(256, NQ)
        mhat[b, 0] = _row_offsets(g0f, g1f)   # task A rows
        mhat[b, 1] = _row_offsets(g1f, g0f)   # task B rows
    p = np.arange(128)
    in_maps = []
    for core in range(NCORES):
        b, kq = divmod(core, 4)
        h0 = 16 * kq + 2 * np.arange(NCHUNK)[None, :] + (p // 64)[:, None]
        by8 = (-(h0.astype(np.float32)) - 31.5).astype(np.float32)
        # negm[t, part, chunk] = -50 * mhat[row = 1024*kq + 128*chunk + part]
        rows = 1024 * kq + 128 * np.arange(NCHUNK)[None, :] + p[:, None]
        negm = (-INV_BETA * mhat[b][:, rows]).astype(np.float32)  # [2,128,8]
        in_maps.append({
            "lhsA": np.ascontiguousarray(g0[b][:, :, kq * 1024 : (kq + 1) * 1024]),
            "rhsA": g1[b],
            "lhsB": np.ascontiguousarray(g1[b][:, :, kq * 1024 : (kq + 1) * 1024]),
            "rhsB": g0[b],
            "wx": wx, "wy": wy, "bx8": bx8, "by8": by8,
            "negm": np.ascontiguousarray(negm),
        })

    import os
    nc = _get_nc()
    trace = bool(int(os.environ.get("BMGC_TRACE", "0")))
    res = run_bass_kernel_spmd(
        nc, in_maps, core_ids=list(range(NCORES)), trace=trace,
        **({"trace_cores": [0]} if trace else {}),
    )
    _cache["last_res"] = res

    c_full = np.empty((B, 4096, NQ), dtype=np.float32)
    flow = np.empty((B, 2, S, S), dtype=np.float32)
    flow_flip = np.empty((B, 2, S, S), dtype=np.float32)
    for core in range(NCORES):
        b, kq = divmod(core, 4)
        r = res.results[core]
        c_full[b, kq * 1024 : (kq + 1) * 1024] = r["c_out"].reshape(1024, NQ)
        fxy = r["fxy"]                                   # [2,2,128,8]
        for ti, dst in ((0, flow), (1, flow_flip)):
            for xi in range(2):
                dst[b, xi, 16 * kq : 16 * (kq + 1), :] = (
                    fxy[ti, xi].T.reshape(16, 64)
                )

    c = c_full.reshape(B, 1, S, S, S, S)
    c_flip = np.ascontiguousarray(
        c_full.reshape(B, S, S, S, S).transpose(0, 3, 4, 1, 2)
    ).reshape(B, 1, S, S, S, S)
    trans_features = np.stack([feat0_f, feat1_f], axis=1)
    trans_features_coarse = np.stack([f0c_up, f1c_up], axis=1)
    return (trans_features, trans_features_coarse, c, c_flip, flow, flow_flip)


# revision 23
# speedup vs baseline: 1.0328x; 1.0328x over previous
"""Trainium2 Bass kernel for BackboneMultiviewGmCoPo.

Math: the 4D bilinear upsample of the coarse cost volume is linear in the
features, so  c = 0.5*(up4d(corr_c) + corr_f) = a * G0^T G1  over stacked
256-channel features G = [resize(F_c); F_f], a = 0.5/sqrt(128).
Sharding: 8 cores = (batch b in {0,1}) x (quarter kq in {0..3}) of the 4096
rows.  Per core, task A computes M[p_slice, :] (c rows + row-softmax ->
flow), task B computes M^T[q_slice, :] (row-softmax -> flow_flip).
c_flip is a host-side transpose view of c; trans_features is a host restack
of the inputs; trans_features_coarse is the (host) bilinear resize that also
feeds the matmul.
"""
import numpy as np
import ml_dtypes

import concourse.bass as bass
import concourse.mybir as mybir
import concourse.tile as tile
from concourse.bass_utils import run_bass_kernel_spmd
from concourse.vector_clock import ScopedClock

# ---------------------------------------------------------------------------
# Walrus-compat shims (this container's walrus accepts at most ONE sem wait
# per instruction): split the Tile tail-drain's waits across single-wait
# Drains, and hoist excess waits of any instruction onto NoOp carriers.
_MAXW = 1
_wsn = [0]


def _patched_drain_and_barrier(self, tick_clock, wait_clock):
    drain_inst = self.nc.sync.drain()
    wait_clock.add_sem_waits(drain_inst.ins, ScopedClock({None: tick_clock.global_clock}))
    si = drain_inst.ins.sync_info
    waits = list(si.on_wait) if si is not None else []
    if len(waits) > _MAXW:
        drain_inst.ins.sync_info = mybir.SyncInfo(on_wait=waits[:_MAXW], on_update=[])
        for i in range(_MAXW, len(waits), _MAXW):
            extra = self.nc.sync.drain()
            extra.ins.sync_info = mybir.SyncInfo(on_wait=waits[i : i + _MAXW], on_update=[])
    self.nc.all_engine_barrier()
    assert self.sems is not None
    popped = self.nc._tile_sem_poison_stack.pop()
    assert popped is self._sem_poison
    self.nc.clear_and_free_semaphores(list(self.sems.allocated().values()))
    self.nc.all_engine_barrier()


tile.TileContext._drain_and_barrier = _patched_drain_and_barrier


def _split_multi_waits(nc):
    NoOp = getattr(mybir, "InstNoOp", None)
    if NoOp is None:
        import bass_rust
        NoOp = bass_rust.InstNoOp
    for fn in nc.m.functions:
        for bb in fn.blocks:
            if not any(
                i.sync_info is not None and len(i.sync_info.on_wait) > _MAXW
                for i in bb.instructions
            ):
                continue
            new_insts = []
            for i in bb.instructions:
                si = i.sync_info
                if si is not None and len(si.on_wait) > _MAXW:
                    waits = list(si.on_wait)
                    for w in waits[:-_MAXW]:
                        _wsn[0] += 1
                        new_insts.append(NoOp(
                            name=f"WSPLIT-{_wsn[0]}", engine=i.engine, ins=[], outs=[],
                            sync_info=mybir.SyncInfo(on_wait=[w], on_update=[]),
                        ))
                    i.sync_info = mybir.SyncInfo(
                        on_wait=waits[-_MAXW:], on_update=list(si.on_update)
                    )
                new_insts.append(i)
            bb.instructions = new_insts
# ---------------------------------------------------------------------------

F32 = mybir.dt.float32
F32R = mybir.dt.float32r
BF16 = mybir.dt.bfloat16
AOP = mybir.AluOpType
AFT = mybir.ActivationFunctionType

B, C, S = 2, 128, 64
NCORES = 8
NQ = S * S            # 4096
NCHUNK = 8            # 128-row chunks per task
HALF = 2048           # free-dim half-chunk (4 PSUM banks)
BETA = 0.02
INV_BETA = 1.0 / BETA  # 50
MARGIN = 0.0          # exact per-half row max → no margin needed
SCALE = float(np.sqrt(0.5 / np.sqrt(128.0)))

_cache = {}


def _resize_ac64(x):
    """Bilinear align_corners resize of last two dims 32x32 -> 64x64 (numpy)."""
    H = x.shape[-1]
    ys = np.linspace(0.0, H - 1.0, 64).astype(np.float32)
    y0 = np.floor(ys).astype(np.int64)
    y1 = np.minimum(y0 + 1, H - 1)
    wy = (ys - y0).astype(np.float32)
    rows = x[..., y0, :] * (1.0 - wy)[:, None] + x[..., y1, :] * wy[:, None]
    return rows[..., y0] * (1.0 - wy) + rows[..., y1] * wy


def _build_nc():
    import os
    mode = os.environ.get("BMGC_MODE", "full")  # full | nodma | dmaonly
    nc = bass.Bass(trn_type="TRN2")
    d = {}
    d["lhsA"] = nc.dram_tensor("lhsA", [2, 128, 1024], F32R, kind="ExternalInput")
    d["rhsA"] = nc.dram_tensor("rhsA", [2, 128, NQ], F32R, kind="ExternalInput")
    d["lhsB"] = nc.dram_tensor("lhsB", [2, 128, 1024], F32R, kind="ExternalInput")
    d["rhsB"] = nc.dram_tensor("rhsB", [2, 128, NQ], F32R, kind="ExternalInput")
    d["wx"] = nc.dram_tensor("wx", [128, NQ], BF16, kind="ExternalInput")
    d["wy"] = nc.dram_tensor("wy", [128, NQ], BF16, kind="ExternalInput")
    d["bx8"] = nc.dram_tensor("bx8", [128, NCHUNK], F32, kind="ExternalInput")
    d["by8"] = nc.dram_tensor("by8", [128, NCHUNK], F32, kind="ExternalInput")
    # host-computed per-row softmax offsets, pre-scaled: -50 * mhat  [task,128,chunk]
    d["negm"] = nc.dram_tensor("negm", [2, 128, NCHUNK], F32, kind="ExternalInput")
    d["c_out"] = nc.dram_tensor("c_out", [NCHUNK, 128, NQ], F32, kind="ExternalOutput")
    d["fxy"] = nc.dram_tensor("fxy", [2, 2, 128, NCHUNK], F32, kind="ExternalOutput")

    with tile.TileContext(nc) as tc:
        with (
            tc.tile_pool(name="const", bufs=1) as constp,
            tc.tile_pool(name="mats", bufs=1) as matp,
            tc.tile_pool(name="msb", bufs=3) as msbp,
            tc.tile_pool(name="ep", bufs=5) as ep,
            tc.tile_pool(name="scr", bufs=2) as scrp,
            tc.tile_pool(name="stat", bufs=2) as statp,
            tc.tile_pool(name="ps", bufs=2, space="PSUM") as psp,
        ):
            # critical-path-first load order: task A matmul operands stream in
            # per-(k, half) pieces so chunk 0 can start after ~3MB, then task B
            # operands, then the softmax constants (first needed ~10us in).
            lhs_t = {}
            rhs_t = {}
            for T, lname, rname in (("A", "lhsA", "rhsA"), ("B", "lhsB", "rhsB")):
                lt = matp.tile([128, 2, 1024], F32R, tag=f"lhs{T}")
                rt = matp.tile([128, 2, NQ], F32R, tag=f"rhs{T}")
                for k in range(2):
                    nc.sync.dma_start(lt[:, k], d[lname][k])
                for half in range(2):
                    for k in range(2):
                        sl = slice(half * HALF, (half + 1) * HALF)
                        nc.sync.dma_start(rt[:, k, sl], d[rname][k][:, sl])
                lhs_t[T], rhs_t[T] = lt, rt

            wx_t = constp.tile([128, NQ], BF16)
            wy_t = constp.tile([128, NQ], BF16)
            bx_t = constp.tile([128, NCHUNK], F32)
            by_t = constp.tile([128, NCHUNK], F32)
            negm_t = constp.tile([128, 2, NCHUNK], F32)
            nc.sync.dma_start(wx_t[:], d["wx"][:])
            nc.sync.dma_start(wy_t[:], d["wy"][:])
            nc.sync.dma_start(bx_t[:], d["bx8"][:])
            nc.sync.dma_start(by_t[:], d["by8"][:])
            nc.sync.dma_start(negm_t[:], d["negm"].rearrange("t p c -> p t c"))

            for ti, T in enumerate(("A", "B")):
                m_a = statp.tile([128, NCHUNK], F32, tag=f"ma{T}")
                m_b = statp.tile([128, NCHUNK], F32, tag=f"mb{T}")
                mc = statp.tile([128, NCHUNK], F32, tag=f"mc{T}")
                negm = statp.tile([128, NCHUNK], F32, tag=f"ng{T}")
                D_h = statp.tile([128, 16], F32, tag=f"D{T}")
                Nx_h = statp.tile([128, 16], F32, tag=f"Nx{T}")
                Ny_h = statp.tile([128, 16], F32, tag=f"Ny{T}")
                def emit_marginals(h, e_t):
                    half = h % 2
                    scx = scrp.tile([128, HALF], BF16, tag="scx")
                    nc.vector.scalar_tensor_tensor(
                        out=scx[:], in0=e_t[:], scalar=1.0,
                        in1=wx_t[:, half * HALF : (half + 1) * HALF],
                        op0=AOP.mult, op1=AOP.mult,
                        accum_out=Nx_h[:, h : h + 1],
                    )
                    scy = scrp.tile([128, HALF], BF16, tag="scy")
                    nc.vector.scalar_tensor_tensor(
                        out=scy[:], in0=e_t[:], scalar=1.0,
                        in1=wy_t[:, half * HALF : (half + 1) * HALF],
                        op0=AOP.mult, op1=AOP.mult,
                        accum_out=Ny_h[:, h : h + 1],
                    )

                pend = []  # deferred (h, e_t) marginals (one chunk of delay)
                for chunk in range(NCHUNK):
                    phs = []
                    esrcs = []
                    for half in range(2):
                        ph = psp.tile([128, HALF], F32, tag="ph")
                        lhs_s = lhs_t[T][:, :, chunk * 128 : (chunk + 1) * 128]
                        for k in range(2):
                            for bank in range(4):
                                q0 = half * HALF + bank * 512
                                nc.tensor.matmul(
                                    ph[:, bank * 512 : (bank + 1) * 512],
                                    lhs_s[:, k],
                                    rhs_t[T][:, k, q0 : q0 + 512],
                                    start=(k == 0),
                                    stop=(k == 1),
                                )
                        phs.append(ph)
                        # per-half exact row max from PSUM
                        mdst = (m_a if half == 0 else m_b)[:, chunk : chunk + 1]
                        nc.vector.tensor_reduce(
                            out=mdst, in_=ph[:], axis=mybir.AxisListType.X, op=AOP.max,
                        )
                        msb = msbp.tile([128, HALF], F32, tag="msb")
                        nc.scalar.activation(
                            msb[:], ph[:], AFT.Copy, bias=0.0, scale=1.0
                        )
                        if T == "A":
                            nc.sync.dma_start(
                                d["c_out"][chunk, :, half * HALF : (half + 1) * HALF],
                                msb[:],
                            )
                        esrcs.append(msb)
                    # one offset per chunk = max of the two halves, pre-scaled
                    nc.vector.tensor_tensor(
                        out=mc[:, chunk : chunk + 1],
                        in0=m_a[:, chunk : chunk + 1], in1=m_b[:, chunk : chunk + 1],
                        op=AOP.max,
                    )
                    nc.scalar.activation(
                        negm[:, chunk : chunk + 1], mc[:, chunk : chunk + 1],
                        AFT.Copy, bias=0.0, scale=-INV_BETA,
                    )
                    for half in range(2):
                        h = chunk * 2 + half
                        e_t = ep.tile([128, HALF], BF16, tag="e")
                        nc.scalar.activation(
                            e_t[:], esrcs[half][:], AFT.Exp,
                            bias=negm[:, chunk : chunk + 1], scale=INV_BETA,
                            accum_out=D_h[:, h : h + 1],
                        )
                        pend.append((h, e_t))
                    while len(pend) > 2:
                        emit_marginals(*pend.pop(0))
                for item in pend:
                    emit_marginals(*item)

                # ---- per-task epilogue: combine halves (same offset per chunk),
                # softargmax -> flow
                ev = lambda t: t[:].rearrange("p (c h) -> p c h", h=2)[:, :, 0]
                od = lambda t: t[:].rearrange("p (c h) -> p c h", h=2)[:, :, 1]
                Dc = statp.tile([128, NCHUNK], F32, tag=f"dc{T}")
                Nxc = statp.tile([128, NCHUNK], F32, tag=f"nxc{T}")
                Nyc = statp.tile([128, NCHUNK], F32, tag=f"nyc{T}")
                nc.vector.tensor_tensor(out=Dc[:], in0=ev(D_h), in1=od(D_h), op=AOP.add)
                nc.vector.tensor_tensor(out=Nxc[:], in0=ev(Nx_h), in1=od(Nx_h), op=AOP.add)
                nc.vector.tensor_tensor(out=Nyc[:], in0=ev(Ny_h), in1=od(Ny_h), op=AOP.add)
                rD = statp.tile([128, NCHUNK], F32, tag=f"rd{T}")
                nc.vector.reciprocal(rD[:], Dc[:])
                fx = statp.tile([128, NCHUNK], F32, tag=f"fx{T}")
                fy = statp.tile([128, NCHUNK], F32, tag=f"fy{T}")
                tx = statp.tile([128, NCHUNK], F32, tag=f"tx{T}")
                ty = statp.tile([128, NCHUNK], F32, tag=f"ty{T}")
                nc.vector.scalar_tensor_tensor(
                    out=tx[:], in0=Nxc[:], scalar=31.5, in1=rD[:],
                    op0=AOP.mult, op1=AOP.mult)
                nc.vector.scalar_tensor_tensor(
                    out=ty[:], in0=Nyc[:], scalar=31.5, in1=rD[:],
                    op0=AOP.mult, op1=AOP.mult)
                nc.vector.tensor_tensor(out=fx[:], in0=tx[:], in1=bx_t[:], op=AOP.add)
                nc.vector.tensor_tensor(out=fy[:], in0=ty[:], in1=by_t[:], op=AOP.add)
                nc.sync.dma_start(d["fxy"][ti, 0], fx[:])
                nc.sync.dma_start(d["fxy"][ti, 1], fy[:])

    _split_multi_waits(nc)
    return nc


def _get_nc():
    if "nc" not in _cache:
        _cache["nc"] = _build_nc()
    return _cache["nc"]


def _host_consts():
    if "consts" in _cache:
        return _cache["consts"]
    xn = np.linspace(-1.0, 1.0, S).astype(np.float32)
    q = np.arange(NQ)
    wxq = (xn[q % S] + 1.0).astype(np.float32)          # shift +1
    wyq = (xn[q // S] + 2.0).astype(np.float32)         # shift +2
    wx = np.broadcast_to(wxq, (128, NQ)).astype(ml_dtypes.bfloat16)
    wy = np.broadcast_to(wyq, (128, NQ)).astype(ml_dtypes.bfloat16)
    p = np.arange(128)
    bx8 = np.broadcast_to((-(p % 64)).astype(np.float32)[:, None], (128, NCHUNK)).copy()
    _cache["consts"] = (wx, wy, bx8)
    return _cache["consts"]


def _row_offsets(ga, gb):
    """Softmax offset estimate per row of M = ga^T gb (ga, gb: [256, 4096]).
    Any value within (truemax - 1.7, truemax + 1.7) is numerically safe; build
    it from the correlated diagonal band + a stride-64 sample + a statistical
    row bound, all input-adaptive."""
    ga64 = ga.astype(np.float64)
    gb64 = gb.astype(np.float64)
    n = ga.shape[1]
    band = np.full(n, -np.inf)
    for dd in (0, 1, -1, 63, 64, 65, -63, -64, -65):
        if dd >= 0:
            v = (ga64[:, : n - dd] * gb64[:, dd:]).sum(0)
            band[: n - dd] = np.maximum(band[: n - dd], v)
        else:
            v = (ga64[:, -dd:] * gb64[:, : n + dd]).sum(0)
            band[-dd:] = np.maximum(band[-dd:], v)
    subm = (ga64.T @ gb64[:, ::64]).max(axis=1)
    sigma = np.sqrt((ga64 * ga64).sum(0)) * np.sqrt((gb64 * gb64).sum(0).mean() / ga.shape[0])
    est = 3.5 * sigma
    return np.maximum(np.maximum(band, subm), est).astype(np.float32)


def kernel(feat0_c, feat1_c, feat0_f, feat1_f):
    feat0_c = np.asarray(feat0_c, dtype=np.float32)
    feat1_c = np.asarray(feat1_c, dtype=np.float32)
    feat0_f = np.asarray(feat0_f, dtype=np.float32)
    feat1_f = np.asarray(feat1_f, dtype=np.float32)

    f0c_up = _resize_ac64(feat0_c)                      # [B,C,64,64]
    f1c_up = _resize_ac64(feat1_c)
    g0 = np.concatenate([f0c_up, feat0_f], axis=1) * SCALE   # [B,256,64,64]
    g1 = np.concatenate([f1c_up, feat1_f], axis=1) * SCALE
    g0 = np.ascontiguousarray(g0.reshape(B, 2, 128, NQ), dtype=np.float32)
    g1 = np.ascontiguousarray(g1.reshape(B, 2, 128, NQ), dtype=np.float32)

    wx, wy, bx8 = _host_consts()
    mhat = np.empty((B, 2, 4096), dtype=np.float32)
    for b in range(B):
        g0f = g0[b].reshape(256, NQ)
        g1f = g1[b].reshape(256, NQ)
        mhat[b, 0] = _row_offsets(g0f, g1f)   # task A rows
        mhat[b, 1] = _row_offsets(g1f, g0f)   # task B rows
    p = np.arange(128)
    in_maps = []
    for core in range(NCORES):
        b, kq = divmod(core, 4)
        h0 = 16 * kq + 2 * np.arange(NCHUNK)[None, :] + (p // 64)[:, None]
        by8 = (-(h0.astype(np.float32)) - 31.5).astype(np.float32)
        # negm[t, part, chunk] = -50 * mhat[row = 1024*kq + 128*chunk + part]
        rows = 1024 * kq + 128 * np.arange(NCHUNK)[None, :] + p[:, None]
        negm = (-INV_BETA * mhat[b][:, rows]).astype(np.float32)  # [2,128,8]
        in_maps.append({
            "lhsA": np.ascontiguousarray(g0[b][:, :, kq * 1024 : (kq + 1) * 1024]),
            "rhsA": g1[b],
            "lhsB": np.ascontiguousarray(g1[b][:, :, kq * 1024 : (kq + 1) * 1024]),
            "rhsB": g0[b],
            "wx": wx, "wy": wy, "bx8": bx8, "by8": by8,
            "negm": np.ascontiguousarray(negm),
        })

    import os
    nc = _get_nc()
    trace = bool(int(os.environ.get("BMGC_TRACE", "0")))
    res = run_bass_kernel_spmd(
        nc, in_maps, core_ids=list(range(NCORES)), trace=trace,
        **({"trace_cores": [0]} if trace else {}),
    )
    _cache["last_res"] = res

    c_full = np.empty((B, 4096, NQ), dtype=np.float32)
    flow = np.empty((B, 2, S, S), dtype=np.float32)
    flow_flip = np.empty((B, 2, S, S), dtype=np.float32)
    for core in range(NCORES):
        b, kq = divmod(core, 4)
        r = res.results[core]
        c_full[b, kq * 1024 : (kq + 1) * 1024] = r["c_out"].reshape(1024, NQ)
        fxy = r["fxy"]                                   # [2,2,128,8]
        for ti, dst in ((0, flow), (1, flow_flip)):
            for xi in range(2):
                dst[b, xi, 16 * kq : 16 * (kq + 1), :] = (
                    fxy[ti, xi].T.reshape(16, 64)
                )

    c = c_full.reshape(B, 1, S, S, S, S)
    c_flip = np.ascontiguousarray(
        c_full.reshape(B, S, S, S, S).transpose(0, 3, 4, 1, 2)
    ).reshape(B, 1, S, S, S, S)
    trans_features = np.stack([feat0_f, feat1_f], axis=1)
    trans_features_coarse = np.stack([f0c_up, f1c_up], axis=1)
    return (trans_features, trans_features_coarse, c, c_flip, flow, flow_flip)
